# revision 6
# baseline (speedup 1.0000x reference)
"""DGCNN forward on 8 TRN2 NeuronCores — pure data parallel over batch.

Layout: feature-major xT[C, N] (channels in partitions, points in free dim).
EdgeConv:  E = max(v, 0.2*v),  v = gs*(max_k A[idx[n,k]] + Bc[n]) + b
with A = x@WnT, Bc = x@(Wc-Wn)T precomputed per layer (max over neighbors
commutes with the monotone per-channel affine + LeakyReLU since gs > 0).

kNN top-20: rank rows of s'[n,j] = <x_n,x_j> - xx_j/2 (same order per row as
the reference's pairwise metric), via DVE max/max_index/match_replace
(3 rounds of 8 -> top-24 values+indices, take first 20 = exact top-20 set).

Gather: per 128-point tile one dma_gather (2560 rows) from a DRAM A-table;
indices are folded into the wrapped-16 int16 layout the instruction expects.
"""
import numpy as np

import concourse.bass as bass
import concourse.mybir as mybir
import concourse.tile as tile
from concourse import bacc
from concourse.masks import make_identity

F32 = mybir.dt.float32
U16 = mybir.dt.uint16
U32 = mybir.dt.uint32
I16 = mybir.dt.int16

KNN = 20
NCORES = 8
CS = [3, 64, 64, 128, 256]     # conv input channels
COS = [64, 64, 128, 256]       # conv output channels
EPS = 1e-5
BN_SCALE = 1.0 / np.sqrt(1.0 + EPS)
NEG = -3.0e38


def _blocks(c):
    out = []
    off = 0
    while off < c:
        b = min(128, c - off)
        out.append((off, b))
        off += b
    return out


def build(N=2048, debug=False):
    """Emit the per-core Bass program. Returns compiled nc."""
    nc = bacc.Bacc("TRN2", target_bir_lowering=False, debug=debug,
                   num_swdge_queues=4)
    NT = N // 128
    CHW = min(N, 512)
    NCH = N // CHW

    # ---------------- DRAM I/O ----------------
    xT_d = nc.dram_tensor("xT", [CS[0], N], F32, kind="ExternalInput")
    wn_d, wd_d, gs_d, bb_d = [], [], [], []
    for l in range(4):
        wn_d.append(nc.dram_tensor(f"WnT{l}", [CS[l], COS[l]], F32, kind="ExternalInput"))
        wd_d.append(nc.dram_tensor(f"WdT{l}", [CS[l], COS[l]], F32, kind="ExternalInput"))
        gs_d.append(nc.dram_tensor(f"gs{l}", [COS[l], 1], F32, kind="ExternalInput"))
        bb_d.append(nc.dram_tensor(f"bb{l}", [COS[l], 1], F32, kind="ExternalInput"))
    w5_d = nc.dram_tensor("W5T", [512, 1024], F32, kind="ExternalInput")
    gs5_d = nc.dram_tensor("gs5", [1024, 1], F32, kind="ExternalInput")
    bb5_d = nc.dram_tensor("bb5", [1024, 1], F32, kind="ExternalInput")
    l1_d = nc.dram_tensor("L1T", [2048, 512], F32, kind="ExternalInput")
    gs6_d = nc.dram_tensor("gs6r", [1, 512], F32, kind="ExternalInput")
    bb6_d = nc.dram_tensor("bb6r", [1, 512], F32, kind="ExternalInput")
    l2_d = nc.dram_tensor("L2T", [512, 256], F32, kind="ExternalInput")
    gs7_d = nc.dram_tensor("gs7r", [1, 256], F32, kind="ExternalInput")
    bb7_d = nc.dram_tensor("bb7r", [1, 256], F32, kind="ExternalInput")
    l3_d = nc.dram_tensor("L3T", [256, 10], F32, kind="ExternalInput")
    l3b_d = nc.dram_tensor("L3br", [1, 10], F32, kind="ExternalInput")
    out_d = nc.dram_tensor("out", [1, 10], F32, kind="ExternalOutput")

    with tile.TileContext(nc) as tc:
        import contextlib
        ctx = contextlib.ExitStack()
        with ctx:
            cpool = ctx.enter_context(tc.tile_pool(name="const", bufs=1))
            xpool = ctx.enter_context(tc.tile_pool(name="xt", bufs=1))
            spool = ctx.enter_context(tc.tile_pool(name="ssb", bufs=2))
            gpool = ctx.enter_context(tc.tile_pool(name="gat", bufs=2))
            wpool = ctx.enter_context(tc.tile_pool(name="work", bufs=2))
            strm = ctx.enter_context(tc.tile_pool(name="strm", bufs=2))
            dram = ctx.enter_context(tc.tile_pool(name="dram", bufs=2, space="DRAM"))
            s_ps = ctx.enter_context(tc.tile_pool(name="s_ps", bufs=2, space="PSUM"))
            t_ps = ctx.enter_context(tc.tile_pool(name="t_ps", bufs=1, space="PSUM"))
            r_ps = ctx.enter_context(tc.tile_pool(name="r_ps", bufs=2, space="PSUM"))

            # ---------------- constants / weights to SBUF ----------------
            ident = cpool.tile([128, 128], F32, tag="ident")
            make_identity(nc, ident[:])
            ones128 = cpool.tile([1, 128], F32, tag="ones128")
            nc.vector.memset(ones128[:], 1.0)
            neghalf = cpool.tile([128, 1], F32, tag="neghalf")
            nc.vector.memset(neghalf[:], -0.5)

            wn_sb, wd_sb, gs_sb, bb_sb = [], [], [], []
            for l in range(4):
                wn_b, wd_b, gs_b, bb_b = [], [], [], []
                for (off, b) in _blocks(CS[l]):
                    t1 = cpool.tile([b, COS[l]], F32, tag=f"wn{l}_{off}")
                    nc.sync.dma_start(t1[:], wn_d[l][off:off + b, :])
                    wn_b.append(t1)
                    t2 = cpool.tile([b, COS[l]], F32, tag=f"wd{l}_{off}")
                    nc.sync.dma_start(t2[:], wd_d[l][off:off + b, :])
                    wd_b.append(t2)
                for (off, b) in _blocks(COS[l]):
                    t3 = cpool.tile([b, 1], F32, tag=f"gs{l}_{off}")
                    nc.sync.dma_start(t3[:], gs_d[l][off:off + b, :])
                    gs_b.append(t3)
                    t4 = cpool.tile([b, 1], F32, tag=f"bb{l}_{off}")
                    nc.sync.dma_start(t4[:], bb_d[l][off:off + b, :])
                    bb_b.append(t4)
                wn_sb.append(wn_b); wd_sb.append(wd_b)
                gs_sb.append(gs_b); bb_sb.append(bb_b)

            w5_rows = [(0, 64), (64, 64), (128, 128), (256, 128), (384, 128)]
            w5_sb = []
            for (off, b) in w5_rows:
                t = cpool.tile([b, 1024], F32, tag=f"w5_{off}")
                nc.sync.dma_start(t[:], w5_d[off:off + b, :])
                w5_sb.append(t)
            gs5_sb, bb5_sb = [], []
            for (off, b) in _blocks(1024):
                t = cpool.tile([b, 1], F32, tag=f"gs5_{off}")
                nc.sync.dma_start(t[:], gs5_d[off:off + b, :]); gs5_sb.append(t)
                t = cpool.tile([b, 1], F32, tag=f"bb5_{off}")
                nc.sync.dma_start(t[:], bb5_d[off:off + b, :]); bb5_sb.append(t)
            l2_sb = []
            for (off, b) in _blocks(512):
                t = cpool.tile([b, 256], F32, tag=f"l2_{off}")
                nc.sync.dma_start(t[:], l2_d[off:off + b, :]); l2_sb.append(t)
            l3_sb = []
            for (off, b) in _blocks(256):
                t = cpool.tile([b, 10], F32, tag=f"l3_{off}")
                nc.sync.dma_start(t[:], l3_d[off:off + b, :]); l3_sb.append(t)
            gs6r = cpool.tile([1, 512], F32, tag="gs6r")
            nc.sync.dma_start(gs6r[:], gs6_d[:])
            bb6r = cpool.tile([1, 512], F32, tag="bb6r")
            nc.sync.dma_start(bb6r[:], bb6_d[:])
            gs7r = cpool.tile([1, 256], F32, tag="gs7r")
            nc.sync.dma_start(gs7r[:], gs7_d[:])
            bb7r = cpool.tile([1, 256], F32, tag="bb7r")
            nc.sync.dma_start(bb7r[:], bb7_d[:])
            l3br = cpool.tile([1, 10], F32, tag="l3br")
            nc.sync.dma_start(l3br[:], l3b_d[:])

            xT0 = xpool.tile([CS[0], N], F32, tag="xT0")
            nc.sync.dma_start(xT0[:], xT_d[:])

            xT_blocks = [xT0]
            saved_xT = []

            # =================== EdgeConv layers ===================
            for l in range(4):
                C, Co = CS[l], COS[l]
                cblks = _blocks(C)
                oblks = _blocks(Co)

                # ---- sqm[j] = -xx_j/2 ----
                xsq = []
                for bi, (off, b) in enumerate(cblks):
                    t = spool.tile([128, N], F32, tag="s_sb")
                    nc.scalar.square(t[:b, :], xT_blocks[bi][:])
                    xsq.append(t)
                sqm = cpool.tile([1, N], F32, tag="sqm")
                for ch in range(NCH):
                    sl = bass.ts(ch, CHW)
                    ps = t_ps.tile([1, CHW], F32, tag="vec_ps")
                    for bi, (off, b) in enumerate(cblks):
                        nc.tensor.matmul(ps[:], lhsT=neghalf[:b, :],
                                         rhs=xsq[bi][:b, sl],
                                         start=(bi == 0), stop=(bi == len(cblks) - 1))
                    nc.scalar.copy(sqm[:, sl], ps[:])

                # ---- A table to DRAM (point-major rows for the gather) ----
                A_dram = dram.tile([N, Co], F32, tag="Adram")
                for t in range(NT):
                    tsl = bass.ts(t, 128)
                    ps = t_ps.tile([128, Co], F32, tag="A_ps")
                    for bi, (off, b) in enumerate(cblks):
                        nc.tensor.matmul(ps[:], lhsT=xT_blocks[bi][:, tsl],
                                         rhs=wn_sb[l][bi][:],
                                         start=(bi == 0), stop=(bi == len(cblks) - 1))
                    asb = wpool.tile([128, Co], F32, tag="A_sb")
                    nc.scalar.copy(asb[:], ps[:])
                    nc.sync.dma_start(A_dram[t * 128:(t + 1) * 128, :], asb[:])

                nxt = [xpool.tile([b, N], F32, tag=f"xT{l + 1}_{off}",
                                  name=f"xT{l + 1}_{off}")
                       for (off, b) in oblks]

                # ---- main per-tile loop ----
                for t in range(NT):
                    tsl = bass.ts(t, 128)
                    s_sb = spool.tile([128, N], F32, tag="s_sb")
                    for ch in range(NCH):
                        sl = bass.ts(ch, CHW)
                        ps = s_ps.tile([128, CHW], F32, tag="s_ps")
                        for bi, (off, b) in enumerate(cblks):
                            nc.tensor.matmul(ps[:], lhsT=xT_blocks[bi][:, tsl],
                                             rhs=xT_blocks[bi][:, sl],
                                             start=(bi == 0), stop=False)
                        nc.tensor.matmul(ps[:], lhsT=ones128[:],
                                         rhs=sqm[:, sl], start=False, stop=True)
                        nc.scalar.copy(s_sb[:, sl], ps[:])

                    v24 = wpool.tile([128, 24], F32, tag="v24")
                    i24 = wpool.tile([128, 24], U32, tag="i24")
                    for r in range(3):
                        nc.vector.max(out=v24[:, 8 * r:8 * r + 8], in_=s_sb[:])
                        nc.vector.max_index(out=i24[:, 8 * r:8 * r + 8],
                                            in_max=v24[:, 8 * r:8 * r + 8],
                                            in_values=s_sb[:])
                        if r < 2:
                            nc.vector.match_replace(out=s_sb[:],
                                                    in_to_replace=v24[:, 8 * r:8 * r + 8],
                                                    in_values=s_sb[:], imm_value=NEG)

                    # gather: one indirect DMA per neighbor rank (per-partition
                    # row index semantics on HW)
                    G = gpool.tile([128, KNN * Co], F32, tag="G")
                    for r in range(KNN):
                        nc.gpsimd.indirect_dma_start(
                            out=G[:, r * Co:(r + 1) * Co], out_offset=None,
                            in_=A_dram[:],
                            in_offset=bass.IndirectOffsetOnAxis(
                                ap=i24[:, r:r + 1], axis=0))

                    M = wpool.tile([128, Co], F32, tag="M")
                    nc.vector.tensor_reduce(out=M[:],
                                            in_=G[:].rearrange("p (r c) -> p c r", r=KNN),
                                            axis=mybir.AxisListType.X,
                                            op=mybir.AluOpType.max)
                    bc = t_ps.tile([128, Co], F32, tag="Bc_ps")
                    for bi, (off, b) in enumerate(cblks):
                        nc.tensor.matmul(bc[:], lhsT=xT_blocks[bi][:, tsl],
                                         rhs=wd_sb[l][bi][:],
                                         start=(bi == 0), stop=(bi == len(cblks) - 1))
                    z = wpool.tile([128, Co], F32, tag="z")
                    nc.vector.tensor_add(z[:], M[:], bc[:])
                    for oi, (off, b) in enumerate(oblks):
                        tp = r_ps.tile([b, 128], F32, tag="tr_ps")
                        nc.tensor.transpose(tp[:], z[:, off:off + b], ident[:])
                        v = wpool.tile([b, 128], F32, tag="v_ep")
                        nc.scalar.activation(v[:], tp[:],
                                             mybir.ActivationFunctionType.Identity,
                                             bias=bb_sb[l][oi][:], scale=gs_sb[l][oi][:])
                        w = wpool.tile([b, 128], F32, tag="w_ep")
                        nc.scalar.mul(w[:], v[:], 0.2)
                        nc.vector.tensor_tensor(out=nxt[oi][:, tsl], in0=v[:], in1=w[:],
                                                op=mybir.AluOpType.max)

                xT_blocks = nxt
                saved_xT.append(nxt)

            # =================== h stage: hmax / havg ===================
            bands = [saved_xT[0][0], saved_xT[1][0], saved_xT[2][0],
                     saved_xT[3][0], saved_xT[3][1]]
            gmax, gavg = [], []
            for mb in range(8):
                msl = bass.ts(mb, 128)
                hmax = wpool.tile([128, 1], F32, tag="hmax")
                sv = wpool.tile([128, 1], F32, tag="sv")
                sa = wpool.tile([128, 1], F32, tag="sa")
                for ch in range(NCH):
                    sl = bass.ts(ch, CHW)
                    ps = s_ps.tile([128, CHW], F32, tag="s_ps")
                    for k in range(5):
                        nc.tensor.matmul(ps[:], lhsT=w5_sb[k][:, msl],
                                         rhs=bands[k][:, sl],
                                         start=(k == 0), stop=(k == 4))
                    v5 = strm.tile([128, CHW], F32, tag="v5")
                    svp = wpool.tile([128, 1], F32, tag="svp")
                    nc.scalar.activation(v5[:], ps[:],
                                         mybir.ActivationFunctionType.Identity,
                                         bias=bb5_sb[mb][:], scale=gs5_sb[mb][:],
                                         accum_out=svp[:])
                    a5 = strm.tile([128, CHW], F32, tag="a5")
                    sap = wpool.tile([128, 1], F32, tag="sap")
                    nc.scalar.activation(a5[:], ps[:],
                                         mybir.ActivationFunctionType.Abs,
                                         bias=bb5_sb[mb][:], scale=gs5_sb[mb][:],
                                         accum_out=sap[:])
                    rmp = wpool.tile([128, 1], F32, tag="rmp")
                    nc.vector.tensor_reduce(out=rmp[:], in_=v5[:],
                                            axis=mybir.AxisListType.X,
                                            op=mybir.AluOpType.max)
                    if ch == 0:
                        nc.vector.tensor_copy(hmax[:], rmp[:])
                        nc.vector.tensor_copy(sv[:], svp[:])
                        nc.vector.tensor_copy(sa[:], sap[:])
                    else:
                        nc.vector.tensor_tensor(hmax[:], hmax[:], rmp[:],
                                                op=mybir.AluOpType.max)
                        nc.vector.tensor_add(sv[:], sv[:], svp[:])
                        nc.vector.tensor_add(sa[:], sa[:], sap[:])
                gm = wpool.tile([128, 1], F32, tag=f"gm{mb}")
                wtmp = wpool.tile([128, 1], F32, tag="wtmp")
                nc.scalar.mul(wtmp[:], hmax[:], 0.2)
                nc.vector.tensor_tensor(gm[:], hmax[:], wtmp[:], op=mybir.AluOpType.max)
                gmax.append(gm)
                ga = wpool.tile([128, 1], F32, tag=f"ga{mb}")
                t1 = wpool.tile([128, 1], F32, tag="t1")
                nc.scalar.mul(t1[:], sv[:], 0.6 / N)
                t2 = wpool.tile([128, 1], F32, tag="t2")
                nc.scalar.mul(t2[:], sa[:], 0.4 / N)
                nc.vector.tensor_add(ga[:], t1[:], t2[:])
                gavg.append(ga)

            gvec = gmax + gavg

            # =================== head (row-vector form) ===================
            def row_affine_lrelu(ps_row, gsr, bbr, width, tag):
                """v = gs*z + b; out = max(v, 0.2v). All [1, width]."""
                v = wpool.tile([1, width], F32, tag=f"{tag}v")
                nc.vector.tensor_mul(v[:], ps_row[:], gsr[:])
                nc.vector.tensor_add(v[:], v[:], bbr[:])
                w = wpool.tile([1, width], F32, tag=f"{tag}w")
                nc.scalar.mul(w[:], v[:], 0.2)
                o = wpool.tile([1, width], F32, tag=f"{tag}o")
                nc.vector.tensor_tensor(o[:], v[:], w[:], op=mybir.AluOpType.max)
                return o

            ps1 = t_ps.tile([1, 512], F32, tag="vec_ps")
            for k in range(16):
                lt = strm.tile([128, 512], F32, tag="l1strm")
                nc.sync.dma_start(lt[:], l1_d[k * 128:(k + 1) * 128, :])
                nc.tensor.matmul(ps1[:], lhsT=gvec[k][:], rhs=lt[:],
                                 start=(k == 0), stop=(k == 15))
            z1r = row_affine_lrelu(ps1, gs6r, bb6r, 512, "z1")

            z1c = []
            for k in range(4):
                tp = r_ps.tile([128, 1], F32, tag="tr_ps")
                nc.tensor.transpose(tp[:], z1r[:, bass.ts(k, 128)], ident[0:1, 0:1])
                c = wpool.tile([128, 1], F32, tag=f"z1c{k}")
                nc.scalar.copy(c[:], tp[:])
                z1c.append(c)

            ps2 = t_ps.tile([1, 256], F32, tag="vec_ps")
            for k in range(4):
                nc.tensor.matmul(ps2[:], lhsT=z1c[k][:], rhs=l2_sb[k][:],
                                 start=(k == 0), stop=(k == 3))
            z2r = row_affine_lrelu(ps2, gs7r, bb7r, 256, "z2")

            z2c = []
            for k in range(2):
                tp = r_ps.tile([128, 1], F32, tag="tr_ps")
                nc.tensor.transpose(tp[:], z2r[:, bass.ts(k, 128)], ident[0:1, 0:1])
                c = wpool.tile([128, 1], F32, tag=f"z2c{k}")
                nc.scalar.copy(c[:], tp[:])
                z2c.append(c)

            ps3 = t_ps.tile([1, 10], F32, tag="vec_ps")
            for k in range(2):
                nc.tensor.matmul(ps3[:], lhsT=z2c[k][:], rhs=l3_sb[k][:],
                                 start=(k == 0), stop=(k == 1))
            osb = wpool.tile([1, 10], F32, tag="osb")
            nc.vector.tensor_add(osb[:], ps3[:], l3br[:])
            nc.sync.dma_start(out_d[:], osb[:])

    nc.compile()
    return nc


def make_in_maps(x, W1, g1, b1, W2, g2, b2, W3, g3, b3, W4, g4, b4,
                 W5, g5, b5, L1, g6, b6, L2, L2b, g7, b7, L3, L3b):
    f = np.float32
    Ws = [np.asarray(W1), np.asarray(W2), np.asarray(W3), np.asarray(W4)]
    gs = [np.asarray(g1), np.asarray(g2), np.asarray(g3), np.asarray(g4)]
    bs = [np.asarray(b1), np.asarray(b2), np.asarray(b3), np.asarray(b4)]
    x = np.asarray(x)
    common = {}
    for l in range(4):
        C = CS[l]
        Wn = Ws[l][:, :C]
        Wd = Ws[l][:, C:] - Wn
        common[f"WnT{l}"] = np.ascontiguousarray(Wn.T, dtype=f)
        common[f"WdT{l}"] = np.ascontiguousarray(Wd.T, dtype=f)
        common[f"gs{l}"] = np.ascontiguousarray((gs[l] * BN_SCALE).reshape(-1, 1), dtype=f)
        common[f"bb{l}"] = np.ascontiguousarray(bs[l].reshape(-1, 1), dtype=f)
    common["W5T"] = np.ascontiguousarray(np.asarray(W5).T, dtype=f)
    common["gs5"] = np.ascontiguousarray((np.asarray(g5) * BN_SCALE).reshape(-1, 1), dtype=f)
    common["bb5"] = np.ascontiguousarray(np.asarray(b5).reshape(-1, 1), dtype=f)
    common["L1T"] = np.ascontiguousarray(np.asarray(L1).T, dtype=f)
    common["gs6r"] = np.ascontiguousarray((np.asarray(g6) * BN_SCALE).reshape(1, -1), dtype=f)
    common["bb6r"] = np.ascontiguousarray(np.asarray(b6).reshape(1, -1), dtype=f)
    common["L2T"] = np.ascontiguousarray(np.asarray(L2).T, dtype=f)
    gs7v = (np.asarray(g7) * BN_SCALE).astype(f)
    common["gs7r"] = np.ascontiguousarray(gs7v.reshape(1, -1), dtype=f)
    common["bb7r"] = np.ascontiguousarray((gs7v * np.asarray(L2b) + np.asarray(b7)).reshape(1, -1), dtype=f)
    common["L3T"] = np.ascontiguousarray(np.asarray(L3).T, dtype=f)
    common["L3br"] = np.ascontiguousarray(np.asarray(L3b).reshape(1, -1), dtype=f)

    in_maps = []
    for i in range(x.shape[0]):
        m = dict(common)
        m["xT"] = np.ascontiguousarray(x[i].T, dtype=f)
        in_maps.append(m)
    return in_maps


_NC_CACHE = {}


def kernel(**inputs):
    from concourse.bass_utils import run_bass_kernel_spmd
    x = np.asarray(inputs["x"])
    B, N, _ = x.shape
    assert B == NCORES
    if N not in _NC_CACHE:
        _NC_CACHE[N] = build(N=N)
    nc = _NC_CACHE[N]
    in_maps = make_in_maps(**inputs)
    res = run_bass_kernel_spmd(nc, in_maps, core_ids=list(range(NCORES)))
    out = np.stack([r["out"].reshape(-1) for r in res.results])
    return out.astype(np.float32)
